# revision 1
# baseline (speedup 1.0000x reference)
"""GCAttention Trainium2 kernel.

Math: in the reference, k = broadcast(gc @ Wk + bk) has identical rows, so
attention scores are constant along the softmax axis -> softmax is exactly
uniform -> attn @ v = mean_n(v) = gc @ Wv + bv (broadcast over tokens).
The whole module therefore reduces to:
    gc   = mean_n x                       (B, C)
    vbar = gc @ Wv + bv                   (B, C)
    ca   = sigmoid(relu(gc@W1+b1)@W2+b2)  (B, C)
    o    = (vbar * ca) @ Wo + bo          (B, C)
    y    = LN(x + o[:,None,:]) * gamma + beta
Sharding: data-parallel over batch B=8 across the 8 cores (1 batch each).

Fast path (the harness case: biases zero, gamma=1, beta=0, checked on host):
LN computed via centered u = x - mu_z + o where mu_z = (rowsum(x)+sum(o))/C,
var = sum(u^2)/C. u is built on DVE (fused stt) or Pool (plain add, mu folded
into the final scale) to balance engines; squares+accum run on ACT.
"""

import numpy as np
import ml_dtypes

B, N, C = 8, 1024, 768
CR = 192
P = 128
NT = N // P   # 8 token tiles per core
KC = C // P   # 6 channel chunks
NH = 2        # free-dim halves for PSUM (384 each)
H = C // NH   # 384
LN_EPS = 1e-5
NCORES = 8
N_DVE_TILES = 3  # phase-2 tiles built on DVE (rest on Pool)
FP8 = True       # fp8-e4m3 GEMV chain (weights + stationaries), scales folded
SW = 16.0        # host weight scale
SG = 16.0        # gc scale
SH = 16.0        # h1 scale
SGT = 32.0       # g scale

_CACHE: dict = {}


def _build(fast: bool):
    from contextlib import ExitStack

    import concourse.bacc as bacc_mod
    import concourse.mybir as mybir
    import concourse.tile as tile

    f32 = mybir.dt.float32
    bf16 = mybir.dt.bfloat16
    fp8 = mybir.dt.float8e4
    use_fp8 = FP8 and fast
    wdt = fp8 if use_fp8 else bf16
    s_gc = (1.0 / N) * (SG if use_fp8 else 1.0)
    s_vb = 1.0 / (SG * SW) if use_fp8 else 1.0
    s_h1 = SH / (SG * SW) if use_fp8 else 1.0
    s_ca = 1.0 / (SH * SW) if use_fp8 else 1.0
    s_g = SGT if use_fp8 else 1.0
    s_o = 1.0 / (SGT * SW) if use_fp8 else 1.0
    AF = mybir.ActivationFunctionType
    OP = mybir.AluOpType
    AX = mybir.AxisListType

    nc = bacc_mod.Bacc("TRN2")
    x = nc.dram_tensor("x", [N, C], f32, kind="ExternalInput")
    wv = nc.dram_tensor("wv", [C, C], wdt, kind="ExternalInput")
    w1 = nc.dram_tensor("w1", [C, CR], wdt, kind="ExternalInput")
    w2 = nc.dram_tensor("w2", [CR, C], wdt, kind="ExternalInput")
    wo = nc.dram_tensor("wo", [C, C], wdt, kind="ExternalInput")
    if not fast:
        # rows: 0=bv 1=b1(padded) 2=b2 3=bo 4=gamma 5=beta
        vecs = nc.dram_tensor("vecs", [1, 6, C], f32, kind="ExternalInput")
    y = nc.dram_tensor("y", [N, C], f32, kind="ExternalOutput")

    with tile.TileContext(nc) as tc, ExitStack() as ctx:
        xp = ctx.enter_context(tc.tile_pool(name="xp", bufs=NT))
        wp = ctx.enter_context(tc.tile_pool(name="wp", bufs=1))
        sm = ctx.enter_context(tc.tile_pool(name="sm", bufs=1))
        up = ctx.enter_context(tc.tile_pool(name="up", bufs=8))
        sq = ctx.enter_context(tc.tile_pool(name="sq", bufs=3))
        st = ctx.enter_context(tc.tile_pool(name="st", bufs=3))
        s8 = ctx.enter_context(tc.tile_pool(name="s8", bufs=NT))
        pp = ctx.enter_context(tc.tile_pool(name="pp", bufs=4, space="PSUM"))
        pc = ctx.enter_context(tc.tile_pool(name="pc", bufs=2, space="PSUM"))
        po = ctx.enter_context(tc.tile_pool(name="po", bufs=2, space="PSUM"))

        # ---- constants ----
        ones_col = sm.tile([P, 1], bf16)
        nc.vector.memset(ones_col, 1.0)
        ones_row = sm.tile([1, P], bf16 if fast else f32)
        nc.vector.memset(ones_row, 1.0)
        one11 = sm.tile([1, 1], wdt)
        nc.vector.memset(one11, 1.0)
        eps_t = sm.tile([P, 1], f32)
        nc.vector.memset(eps_t, LN_EPS)
        # warm the sigmoid activation table early (overlaps with DMA)
        dmy = sm.tile([1, 1], f32)
        nc.vector.memset(dmy, 0.0)
        dmy2 = sm.tile([1, 1], f32)
        nc.scalar.activation(dmy2, dmy, AF.Sigmoid)

        # ---- input DMAs: x first on SP FIFO, then weights in use order ----
        x_sb = []
        for t in range(NT):
            xt = xp.tile([P, C], f32, tag="x_sb", name=f"x_sb{t}")
            nc.sync.dma_start(out=xt, in_=x[t * P : (t + 1) * P, :])
            x_sb.append(xt)
        w1_sb = wp.tile([P, KC, CR], wdt)
        nc.sync.dma_start(out=w1_sb, in_=w1.rearrange("(a p) c -> p a c", p=P))
        w2_sb = wp.tile([P, 2, C], wdt)
        nc.sync.dma_start(out=w2_sb[:, 0, :], in_=w2[0:P, :])
        nc.sync.dma_start(out=w2_sb[0:64, 1, :], in_=w2[P:CR, :])
        wv_sb = wp.tile([P, KC, C], wdt)
        nc.sync.dma_start(out=wv_sb, in_=wv.rearrange("(a p) c -> p a c", p=P))
        wo_sb = wp.tile([P, KC, C], wdt)
        nc.sync.dma_start(out=wo_sb, in_=wo.rearrange("(a p) c -> p a c", p=P))
        if not fast:
            vec_sb = sm.tile([1, 6, C], f32)
            nc.sync.dma_start(out=vec_sb, in_=vecs[:, :, :])

        # ---- per-tile: bf16 cast (ACT) for column sums; row sums (DVE) ----
        xb_sb = []
        xsum_all = sm.tile([P, NT], f32, name="xsum_all") if fast else None
        for t in range(NT):
            xb = xp.tile([P, C], bf16, tag="xb_sb", name=f"xb_sb{t}")
            if fast:
                if t == NT - 1:
                    # last tile gates gc: cast on DVE (2x mode, faster)
                    nc.vector.tensor_scalar(
                        xb, x_sb[t], 1.0, 0.0, op0=OP.mult, op1=OP.add,
                        accum_out=xsum_all[:, t : t + 1],
                    )
                else:
                    nc.scalar.activation(
                        xb, x_sb[t], AF.Copy, accum_out=xsum_all[:, t : t + 1]
                    )
            else:
                nc.scalar.copy(xb, x_sb[t])
            xb_sb.append(xb)

        # ---- gc = mean_n x  (column sums via PE; ones stationary) ----
        cs_ps = [pp.tile([1, H], f32, tag="rowps", name=f"cs_ps{h}") for h in range(NH)]
        for t in range(NT):
            for h in range(NH):
                nc.tensor.matmul(
                    cs_ps[h],
                    ones_col,
                    xb_sb[t][:, h * H : (h + 1) * H],
                    start=(t == 0),
                    stop=(t == NT - 1),
                )
        gc_sb = sm.tile([1, C], wdt)
        nc.scalar.activation(gc_sb[0:1, 0:H], cs_ps[0], AF.Copy, bias=0.0, scale=s_gc)
        nc.vector.tensor_scalar_mul(gc_sb[0:1, H : 2 * H], cs_ps[1], s_gc)

        # ---- transpose gc to partitions: gcT[:, j] = gc[128j:128j+128] ----
        gcT_ps = pc.tile([P, 8], f32, tag="colps")
        for j in range(KC):
            nc.tensor.matmul(
                gcT_ps[:, j : j + 1], gc_sb[0:1, j * P : (j + 1) * P], one11,
                start=True, stop=True,
            )
        gcT_sb = sm.tile([P, KC], wdt)
        nc.vector.tensor_copy(gcT_sb, gcT_ps[:, 0:KC])

        # ---- h1 = relu(gc @ W1 (+ b1)) ----
        h1_ps = pp.tile([1, CR], f32, tag="rowps")
        for j in range(KC):
            nc.tensor.matmul(
                h1_ps, gcT_sb[:, j : j + 1], w1_sb[:, j, :],
                start=(j == 0), stop=(j == KC - 1),
            )
        vb_ps = [pp.tile([1, H], f32, tag="rowps", name=f"vb_ps{h}") for h in range(NH)]
        for j in range(KC):
            nc.tensor.matmul(
                vb_ps[0],
                gcT_sb[:, j : j + 1],
                wv_sb[:, j, 0:H],
                start=(j == 0),
                stop=(j == KC - 1),
            )
        h1r = sm.tile([1, CR], wdt)
        if fast:
            nc.scalar.activation(h1r, h1_ps, AF.Relu, scale=s_h1)
        else:
            h1f = sm.tile([1, CR], f32)
            nc.vector.tensor_add(h1f, h1_ps, vec_sb[0:1, 1, 0:CR])
            nc.vector.tensor_scalar_max(h1r, h1f, 0.0)

        # ---- transpose h1 (192 = 128 + 64) ----
        h1T_ps = pc.tile([P, 8], f32, tag="colps")
        nc.tensor.matmul(h1T_ps[:, 0:1], h1r[0:1, 0:P], one11, start=True, stop=True)
        nc.tensor.matmul(
            h1T_ps[0:64, 1:2], h1r[0:1, P:CR], one11, start=True, stop=True
        )
        h1T_sb = sm.tile([P, 2], wdt)
        nc.vector.tensor_copy(h1T_sb[:, 0:1], h1T_ps[:, 0:1])
        nc.vector.tensor_copy(h1T_sb[0:64, 1:2], h1T_ps[0:64, 1:2])

        # ---- ca = sigmoid(h1 @ W2 (+ b2)) ----
        ca_ps = [pp.tile([1, H], f32, tag="rowps", name=f"ca_ps{h}") for h in range(NH)]
        for h in range(NH):
            sl = slice(h * H, (h + 1) * H)
            nc.tensor.matmul(
                ca_ps[h], h1T_sb[:, 0:1], w2_sb[:, 0, sl], start=True, stop=False
            )
            nc.tensor.matmul(
                ca_ps[h], h1T_sb[0:64, 1:2], w2_sb[0:64, 1, sl],
                start=False, stop=True,
            )
        ca_sb = sm.tile([1, C], f32)
        if fast:
            for h in range(NH):
                sl = slice(h * H, (h + 1) * H)
                nc.scalar.activation(ca_sb[0:1, sl], ca_ps[h], AF.Sigmoid, scale=s_ca)
        else:
            cap_f = sm.tile([1, C], f32)
            for h in range(NH):
                sl = slice(h * H, (h + 1) * H)
                nc.vector.tensor_add(cap_f[0:1, sl], ca_ps[h], vec_sb[0:1, 2, sl])
            nc.scalar.activation(ca_sb, cap_f, AF.Sigmoid)

        # ---- vbar = gc @ Wv (+ bv) ----
        vbar_sb = sm.tile([1, C], f32)
        if fast:
            nc.scalar.activation(
                vbar_sb[0:1, 0:H], vb_ps[0], AF.Copy, bias=0.0, scale=s_vb
            )
        else:
            nc.vector.tensor_add(vbar_sb[0:1, 0:H], vb_ps[0], vec_sb[0:1, 0, 0:H])

        for j in range(KC):
            nc.tensor.matmul(
                vb_ps[1],
                gcT_sb[:, j : j + 1],
                wv_sb[:, j, H : 2 * H],
                start=(j == 0),
                stop=(j == KC - 1),
            )
        if fast:
            nc.vector.tensor_scalar_mul(vbar_sb[0:1, H : 2 * H], vb_ps[1], s_vb)
        else:
            nc.vector.tensor_add(
                vbar_sb[0:1, H : 2 * H], vb_ps[1], vec_sb[0:1, 0, H : 2 * H]
            )

        # ---- g = vbar * ca ; transpose ; o = g @ Wo — in halves ----
        g_sb = sm.tile([1, C], wdt)
        gT_ps = pc.tile([P, 8], f32, tag="colps")
        gT_sb = sm.tile([P, KC], wdt)
        o_ps = [pp.tile([1, H], f32, tag="rowps", name=f"o_ps{h}") for h in range(NH)]
        for half in range(2):
            hs = slice(half * H, (half + 1) * H)
            if use_fp8:
                nc.vector.scalar_tensor_tensor(
                    out=g_sb[0:1, hs], in0=vbar_sb[0:1, hs], scalar=s_g,
                    in1=ca_sb[0:1, hs], op0=OP.mult, op1=OP.mult,
                )
            else:
                nc.vector.tensor_mul(
                    g_sb[0:1, hs], vbar_sb[0:1, hs], ca_sb[0:1, hs]
                )
            for j in range(half * 3, half * 3 + 3):
                nc.tensor.matmul(
                    gT_ps[:, j : j + 1], g_sb[0:1, j * P : (j + 1) * P], one11,
                    start=True, stop=True,
                )
            nc.vector.tensor_copy(
                gT_sb[:, half * 3 : half * 3 + 3],
                gT_ps[:, half * 3 : half * 3 + 3],
            )
            for j in range(half * 3, half * 3 + 3):
                for h in range(NH):
                    nc.tensor.matmul(
                        o_ps[h],
                        gT_sb[:, j : j + 1],
                        wo_sb[:, j, h * H : (h + 1) * H],
                        start=(j == 0),
                        stop=(j == KC - 1),
                    )

        o_sb = sm.tile([1, C], bf16 if fast else f32)
        if fast:
            nc.scalar.activation(o_sb[0:1, 0:H], o_ps[0], AF.Copy, bias=0.0, scale=s_o)
            nc.vector.tensor_scalar_mul(o_sb[0:1, H : 2 * H], o_ps[1], s_o)
        else:
            for h in range(NH):
                sl = slice(h * H, (h + 1) * H)
                nc.vector.tensor_add(o_sb[0:1, sl], o_ps[h], vec_sb[0:1, 3, sl])

        # ---- broadcast o across partitions via K=1 matmul ----
        obc_sb = sm.tile([P, C], f32)
        for h in range(NH):
            sl = slice(h * H, (h + 1) * H)
            obp = po.tile([P, H], f32, tag="obc", name=f"obp{h}")
            nc.tensor.matmul(obp, ones_row, o_sb[0:1, sl], start=True, stop=True)
            nc.scalar.copy(obc_sb[:, sl], obp)

        if fast:
            # sum(o) on one partition, then broadcast to (128,1) via K=1 matmul
            osum_row = sm.tile([1, 1], f32)
            nc.vector.tensor_reduce(osum_row, o_sb, AX.X, OP.add)
            osum_row_b = sm.tile([1, 1], bf16)
            nc.vector.tensor_copy(osum_row_b, osum_row)
            osb_ps = po.tile([P, 1], f32, tag="obc")
            nc.tensor.matmul(osb_ps, ones_row, osum_row_b, start=True, stop=True)
            osum = sm.tile([P, 1], f32)
            nc.vector.tensor_copy(osum, osb_ps)

            # batched per-tile stats: mu, mu^2, (eps - mu^2)
            mu_all = sm.tile([P, NT], f32)
            nc.vector.tensor_scalar(
                mu_all, xsum_all, osum, 1.0 / C, op0=OP.add, op1=OP.mult
            )
            musq_all = sm.tile([P, NT], f32)
            nc.vector.tensor_mul(musq_all, mu_all, mu_all)
            em_all = sm.tile([P, NT], f32)
            nc.vector.tensor_scalar(
                em_all, musq_all, -1.0, LN_EPS, op0=OP.mult, op1=OP.add
            )

            for t in range(NT):
                on_dve = t >= NT - N_DVE_TILES
                u = up.tile([P, C], f32, tag="u", name=f"u{t}")
                # u = x + o on DVE or Pool; mu folded into the final scale op
                if on_dve:
                    nc.vector.scalar_tensor_tensor(
                        out=u, in0=x_sb[t], scalar=0.0, in1=obc_sb,
                        op0=OP.bypass, op1=OP.add,
                    )
                else:
                    nc.gpsimd.tensor_add(u, x_sb[t], obc_sb)
                usq = sq.tile([P, C], f32, tag="usq")
                uss = s8.tile([P, 1], f32, tag="uss", name=f"uss{t}")
                nc.scalar.activation(usq, u, AF.Square, accum_out=uss)
                # std = sqrt(uss/C + eps - mu^2)
                std = st.tile([P, 1], f32, tag="std")
                nc.scalar.activation(
                    std, uss, AF.Sqrt, bias=em_all[:, t : t + 1], scale=1.0 / C
                )
                rstd = st.tile([P, 1], f32, tag="rstd")
                nc.vector.reciprocal(rstd, std)
                nc.vector.tensor_scalar(
                    u, u, mu_all[:, t : t + 1], rstd, op0=OP.subtract, op1=OP.mult
                )
                nc.scalar.dma_start(out=y[t * P : (t + 1) * P, :], in_=u)
        else:
            gamma_bc = sm.tile([P, C], f32)
            beta_bc = sm.tile([P, C], f32)
            for h in range(NH):
                sl = slice(h * H, (h + 1) * H)
                gbp = po.tile([P, H], f32, tag="obc", name=f"gbp{h}")
                nc.tensor.matmul(
                    gbp, ones_row, vec_sb[0:1, 4, sl], start=True, stop=True
                )
                nc.vector.tensor_copy(gamma_bc[:, sl], gbp)
            for h in range(NH):
                sl = slice(h * H, (h + 1) * H)
                bbp = po.tile([P, H], f32, tag="obc", name=f"bbp{h}")
                nc.tensor.matmul(
                    bbp, ones_row, vec_sb[0:1, 5, sl], start=True, stop=True
                )
                nc.vector.tensor_copy(beta_bc[:, sl], bbp)

            for t in range(NT):
                z = x_sb[t]
                nc.gpsimd.tensor_add(z, z, obc_sb)
                stats = sq.tile([P, 3, 6], f32, tag="stats", name=f"stats{t}")
                zg = z.rearrange("p (s d) -> p s d", s=3)
                for s in range(3):
                    nc.vector.bn_stats(stats[:, s, :], zg[:, s, :])
                mv = st.tile([P, 2], f32, tag="mv")
                nc.vector.bn_aggr(mv, stats)
                std = st.tile([P, 1], f32, tag="std")
                nc.scalar.activation(std, mv[:, 1:2], AF.Sqrt, bias=eps_t)
                rstd = st.tile([P, 1], f32, tag="rstd")
                nc.vector.reciprocal(rstd, std)
                zq = up.tile([P, C], f32, tag="u")
                nc.vector.scalar_tensor_tensor(
                    out=zq, in0=z, scalar=mv[:, 0:1], in1=gamma_bc,
                    op0=OP.subtract, op1=OP.mult,
                )
                nc.vector.tensor_scalar_mul(zq, zq, rstd)
                nc.vector.tensor_add(zq, zq, beta_bc)
                nc.scalar.dma_start(out=y[t * P : (t + 1) * P, :], in_=zq)

    nc.compile()
    return nc


def _get_nc(fast: bool):
    key = ("nc", fast)
    if key not in _CACHE:
        _CACHE[key] = _build(fast)
    return _CACHE[key]


def make_in_maps(x, Wv, bv, W1, b1, W2, b2, Wo, bo, gamma, beta, fast=True):
    if FP8 and fast:
        import concourse.mybir as mybir

        wdt = mybir.dt.np(mybir.dt.float8e4)
        s = SW
    else:
        wdt = ml_dtypes.bfloat16
        s = 1.0
    shared = {
        "wv": np.ascontiguousarray((np.asarray(Wv, np.float32) * s).astype(wdt)),
        "w1": np.ascontiguousarray((np.asarray(W1, np.float32) * s).astype(wdt)),
        "w2": np.ascontiguousarray((np.asarray(W2, np.float32) * s).astype(wdt)),
        "wo": np.ascontiguousarray((np.asarray(Wo, np.float32) * s).astype(wdt)),
    }
    if not fast:
        b1p = np.zeros(C, np.float32)
        b1p[:CR] = np.asarray(b1, np.float32)
        vecs = np.stack(
            [
                np.asarray(bv, np.float32),
                b1p,
                np.asarray(b2, np.float32),
                np.asarray(bo, np.float32),
                np.asarray(gamma, np.float32),
                np.asarray(beta, np.float32),
            ]
        )
        shared["vecs"] = np.ascontiguousarray(vecs.reshape(1, 6, C))
    return [
        {"x": np.ascontiguousarray(np.asarray(x[i], np.float32)), **shared}
        for i in range(NCORES)
    ]


def _is_fast(inputs):
    def z(a):
        return bool(np.all(np.asarray(a) == 0.0))

    return (
        bool(np.all(np.asarray(inputs["gamma"]) == 1.0))
        and z(inputs["beta"]) and z(inputs["bv"]) and z(inputs["b1"])
        and z(inputs["b2"]) and z(inputs["bo"])
    )


def run(inputs, trace=False, **kw):
    from concourse.bass_utils import run_bass_kernel_spmd

    fast = _is_fast(inputs)
    nc = _get_nc(fast)
    in_maps = make_in_maps(
        inputs["x"], inputs["Wv"], inputs["bv"], inputs["W1"], inputs["b1"],
        inputs["W2"], inputs["b2"], inputs["Wo"], inputs["bo"],
        inputs["gamma"], inputs["beta"], fast=fast,
    )
    res = run_bass_kernel_spmd(nc, in_maps, list(range(NCORES)), trace=trace, **kw)
    out = np.stack([r["y"] for r in res.results]).astype(np.float32)
    return out, res


def kernel(
    x, Wq, bq, Wk, bk, Wv, bv, W1, b1, W2, b2, Wo, bo, gamma, beta
) -> np.ndarray:
    # Wq/bq/Wk/bk provably do not affect the output (uniform softmax).
    out, _ = run(
        dict(
            x=x, Wv=Wv, bv=bv, W1=W1, b1=b1, W2=W2, b2=b2, Wo=Wo, bo=bo,
            gamma=gamma, beta=beta,
        )
    )
    return out



# revision 19
# speedup vs baseline: 1.1275x; 1.1275x over previous
"""GCAttention Trainium2 kernel.

Math: in the reference, k = broadcast(gc @ Wk + bk) has identical rows, so
attention scores are constant along the softmax axis -> softmax is exactly
uniform -> attn @ v = mean_n(v) = gc @ Wv + bv (broadcast over tokens).
The whole module therefore reduces to:
    gc   = mean_n x                       (B, C)
    vbar = gc @ Wv + bv                   (B, C)
    ca   = sigmoid(relu(gc@W1+b1)@W2+b2)  (B, C)
    o    = (vbar * ca) @ Wo + bo          (B, C)
    y    = LN(x + o[:,None,:]) * gamma + beta
Sharding: data-parallel over batch B=8 across the 8 cores (1 batch each).

Fast path: all GEMVs are column-oriented on the PE (outputs land as PSUM
columns [128,1] per chunk), the column sums for gc are f32r matmuls with the
x tiles as stationary, and the LN phase overlaps the y writeback DMAs with
adds on DVE/Pool, squares on ACT/DVE, rstd on ACT (Rsqrt), finals on DVE.
"""

import numpy as np
import ml_dtypes

B, N, C = 8, 1024, 768
CR = 192
P = 128
NT = N // P   # 8 token tiles per core
KC = C // P   # 6 channel chunks
H = 384
LN_EPS = 1e-5
NCORES = 8
FP8 = True
SW = 16.0  # host weight scale
SG = 16.0  # gc scale
SH = 16.0  # h1 scale
SGT = 32.0  # g scale

_CACHE: dict = {}


def _build_fast():
    from contextlib import ExitStack

    import concourse.bacc as bacc_mod
    import concourse.mybir as mybir
    import concourse.tile as tile

    f32 = mybir.dt.float32
    f32r = mybir.dt.float32r
    bf16 = mybir.dt.bfloat16
    fp8 = mybir.dt.float8e4
    wdt = fp8 if FP8 else bf16
    s_gc = (1.0 / N) * (SG if FP8 else 1.0)
    s_h1 = SH / (SG * SW) if FP8 else 1.0
    s_ca = 1.0 / (SH * SW) if FP8 else 1.0
    s_vb = 1.0 / (SG * SW) if FP8 else 1.0
    s_g = SGT if FP8 else 1.0
    s_o = 1.0 / (SGT * SW) if FP8 else 1.0
    AF = mybir.ActivationFunctionType
    OP = mybir.AluOpType
    AX = mybir.AxisListType

    nc = bacc_mod.Bacc("TRN2")
    x = nc.dram_tensor("x", [N, C], f32, kind="ExternalInput")
    w1 = nc.dram_tensor("w1", [P, KC, CR], wdt, kind="ExternalInput")
    w2 = nc.dram_tensor("w2", [P, 2, C], wdt, kind="ExternalInput")
    ident = nc.dram_tensor("ident", [P, P], bf16, kind="ExternalInput")
    wv = nc.dram_tensor("wv", [P, KC, C], wdt, kind="ExternalInput")
    wo = nc.dram_tensor("wo", [P, KC, C], wdt, kind="ExternalInput")
    y = nc.dram_tensor("y", [N, C], f32, kind="ExternalOutput")

    # LN engine assignment per tile. Adds for 0-2 and 7 on DVE (reading obc
    # straight from PSUM, in halves), 3-6 on Pool (from the SBUF copy).
    # Squares: tiles 0/1 split ACT||DVE to shorten the first writeback's
    # latency; 2-5 whole on ACT; 6/7 whole on DVE (AFFINE_MUL_REDUCE).
    POOL_ADD = {3, 4, 5, 6}
    SPLIT_SQ = {0, 1}
    DVE_SQ = {6, 7}

    with tile.TileContext(nc) as tc, ExitStack() as ctx:
        xp = ctx.enter_context(tc.tile_pool(name="xp", bufs=NT))
        wp = ctx.enter_context(tc.tile_pool(name="wp", bufs=1))
        sm = ctx.enter_context(tc.tile_pool(name="sm", bufs=1))
        up = ctx.enter_context(tc.tile_pool(name="up", bufs=NT))
        sq = ctx.enter_context(tc.tile_pool(name="sq", bufs=3))
        # PSUM: pc = chain columns (1 bank), pr = o_row (1), pb = obc (2)
        pc = ctx.enter_context(tc.tile_pool(name="pc", bufs=1, space="PSUM"))
        pr = ctx.enter_context(tc.tile_pool(name="pr", bufs=1, space="PSUM"))
        pb = ctx.enter_context(tc.tile_pool(name="pb", bufs=1, space="PSUM"))

        # ---- constants ----
        ones_colr = sm.tile([P, 1], f32)
        nc.vector.memset(ones_colr, 1.0)
        ones_colb = sm.tile([P, 1], bf16)
        nc.vector.memset(ones_colb, 1.0)
        ones_rowb = sm.tile([1, P], bf16)
        nc.vector.memset(ones_rowb, 1.0)
        # warm the sigmoid table early (set also holds relu/copy/square)
        dmy = sm.tile([1, 1], f32)
        nc.vector.memset(dmy, 1.0)
        dmy2 = sm.tile([1, 1], f32)
        nc.scalar.activation(dmy2, dmy, AF.Sigmoid)

        # ---- input DMAs: x first, then weights in chain-use order ----
        x_sb = []
        for t in range(NT):
            xt = xp.tile([P, C], f32, tag="x_sb", name=f"x_sb{t}")
            nc.sync.dma_start(out=xt, in_=x[t * P : (t + 1) * P, :])
            x_sb.append(xt)
        w1_sb = wp.tile([P, KC, CR], wdt)
        nc.sync.dma_start(out=w1_sb, in_=w1[:, :, :])
        w2_sb = wp.tile([P, 2, C], wdt)
        nc.sync.dma_start(out=w2_sb, in_=w2[:, :, :])
        id_sb = wp.tile([P, P], bf16)
        nc.sync.dma_start(out=id_sb, in_=ident[:, :])
        wv_sb = wp.tile([P, KC, C], wdt)
        nc.sync.dma_start(out=wv_sb, in_=wv[:, :, :])
        wo_sb = wp.tile([P, KC, C], wdt)
        nc.sync.dma_start(out=wo_sb, in_=wo[:, :, :])

        # ---- per tile: rowsums (DVE) + transposed colsums (PE, f32r) ----
        xsum_all = sm.tile([P, NT], f32)
        gcT_ps = pc.tile([P, 8], f32, tag="colA")
        for t in range(NT):
            nc.vector.tensor_reduce(
                xsum_all[:, t : t + 1], x_sb[t], AX.X, OP.add
            )
        for j in range(KC):
            for t in range(NT):
                nc.tensor.matmul(
                    gcT_ps[:, j : j + 1],
                    x_sb[t][:, j * P : (j + 1) * P],
                    ones_colr,
                    start=(t == 0),
                    stop=(t == NT - 1),
                )

        # ---- gcT -> SBUF fp8 (scaled) ----
        gcT_sb = sm.tile([P, KC], wdt)
        nc.vector.tensor_scalar_mul(gcT_sb, gcT_ps[:, 0:KC], s_gc)

        # ---- chain columns tile: h1T(0:2) caT(2:8) vbarT(8:14) oT(14:20) ----
        ch_ps = pc.tile([P, 20], f32, tag="colB")

        # h1T = W1.T @ gc  (m-chunks: 128 + 64)
        for j in range(KC):
            nc.tensor.matmul(
                ch_ps[:, 0:1], w1_sb[:, j, 0:P], gcT_sb[:, j : j + 1],
                start=(j == 0), stop=(j == KC - 1),
            )
        for j in range(KC):
            nc.tensor.matmul(
                ch_ps[0:64, 1:2], w1_sb[:, j, P:CR], gcT_sb[:, j : j + 1],
                start=(j == 0), stop=(j == KC - 1),
            )
        # relu + scale -> fp8
        h1T_sb = sm.tile([P, 2], wdt)
        nc.vector.tensor_scalar(
            h1T_sb[:, 0:1], ch_ps[:, 0:1], s_h1, 0.0, op0=OP.mult, op1=OP.max
        )
        nc.vector.tensor_scalar(
            h1T_sb[0:64, 1:2], ch_ps[0:64, 1:2], s_h1, 0.0, op0=OP.mult, op1=OP.max
        )

        # caT = W2.T @ h1 (k: 128 + 64)
        for m in range(KC):
            nc.tensor.matmul(
                ch_ps[:, 2 + m : 3 + m],
                w2_sb[:, 0, m * P : (m + 1) * P],
                h1T_sb[:, 0:1],
                start=True, stop=False,
            )
            nc.tensor.matmul(
                ch_ps[:, 2 + m : 3 + m],
                w2_sb[0:64, 1, m * P : (m + 1) * P],
                h1T_sb[0:64, 1:2],
                start=False, stop=True,
            )
        caT_sb = sm.tile([P, KC], f32)
        nc.scalar.activation(caT_sb, ch_ps[:, 2:8], AF.Sigmoid, scale=s_ca)
        # force the ACT table switch (sigmoid set -> rsqrt/square set) to
        # happen now, inside the wv/wo DMA-wait window, not on the LN path
        dmy3 = sm.tile([1, 1], f32)
        nc.scalar.activation(dmy3, dmy, AF.Sqrt)

        # vbarT = Wv.T @ gc
        for m in range(KC):
            for j in range(KC):
                nc.tensor.matmul(
                    ch_ps[:, 8 + m : 9 + m],
                    wv_sb[:, j, m * P : (m + 1) * P],
                    gcT_sb[:, j : j + 1],
                    start=(j == 0), stop=(j == KC - 1),
                )
        # gT = (vbar * s) * ca -> fp8
        gT_sb = sm.tile([P, KC], wdt)
        nc.vector.scalar_tensor_tensor(
            out=gT_sb, in0=ch_ps[:, 8:14], scalar=s_g * s_vb, in1=caT_sb,
            op0=OP.mult, op1=OP.mult,
        )

        # oT = Wo.T @ g
        for m in range(KC):
            for j in range(KC):
                nc.tensor.matmul(
                    ch_ps[:, 14 + m : 15 + m],
                    wo_sb[:, j, m * P : (m + 1) * P],
                    gT_sb[:, j : j + 1],
                    start=(j == 0), stop=(j == KC - 1),
                )
        # oT -> SBUF bf16 (scaled to real units)
        oT_sb = sm.tile([P, KC], bf16)
        nc.vector.tensor_scalar_mul(oT_sb, ch_ps[:, 14:20], s_o)

        # ---- o_row[0, :] = o (transpose each oT column); broadcast to obc ----
        orow_ps = pr.tile([1, C], bf16, tag="orow", name="orow_ps")
        for m in range(KC):
            nc.tensor.transpose(
                orow_ps[0:1, m * P : (m + 1) * P], oT_sb[:, m : m + 1], id_sb
            )
        orow_sb = sm.tile([1, C], bf16)
        nc.vector.tensor_copy(orow_sb, orow_ps)
        obc_ps = []
        for h in range(2):
            ob = pb.tile([P, H], f32, tag=f"obc{h}", name=f"obc_ps{h}")
            for m in range(3):
                nc.tensor.matmul(
                    ob[:, m * P : (m + 1) * P],
                    ones_rowb,
                    orow_sb[0:1, (3 * h + m) * P : (3 * h + m + 1) * P],
                    start=True, stop=True,
                )
            obc_ps.append(ob)
        # SBUF copy for Pool adds (DVE-add tiles read obc from PSUM directly)
        obc_sb = sm.tile([P, C], f32)
        nc.scalar.copy(obc_sb[:, 0:H], obc_ps[0])
        nc.vector.tensor_copy(obc_sb[:, H:C], obc_ps[1])

        # ---- osum = sum(o): column-sum oT then row-reduce ----
        os_ps = pr.tile([1, 8], f32, tag="osps", name="os_ps")
        nc.tensor.matmul(os_ps[0:1, 0:KC], ones_colb, oT_sb, start=True, stop=True)
        osum_row = sm.tile([1, 1], f32)
        nc.vector.tensor_reduce(osum_row, os_ps[:, 0:KC], AX.X, OP.add)
        osum_rb = sm.tile([1, 1], bf16)
        nc.vector.tensor_copy(osum_rb, osum_row)
        osb_ps = pr.tile([P, 1], f32, tag="osps", name="osb_ps")
        nc.tensor.matmul(osb_ps, ones_rowb, osum_rb, start=True, stop=True)
        osum = sm.tile([P, 1], f32)
        nc.vector.tensor_copy(osum, osb_ps)

        # batched stats: mu, eps - mu^2
        mu_all = sm.tile([P, NT], f32)
        nc.vector.tensor_scalar(
            mu_all, xsum_all, osum, 1.0 / C, op0=OP.add, op1=OP.mult
        )
        musq_all = sm.tile([P, NT], f32)
        nc.vector.tensor_mul(musq_all, mu_all, mu_all)
        em_all = sm.tile([P, NT], f32)
        nc.vector.tensor_scalar(
            em_all, musq_all, -1.0, LN_EPS, op0=OP.mult, op1=OP.add
        )

        # ---- LN phase ----
        ssq_all = sm.tile([P, NT, 2], f32)
        emc_all = sm.tile([P, NT], f32)
        std_all = sm.tile([P, NT], f32)
        rstd_all = sm.tile([P, NT], f32)
        u_t, usq_t = [], []
        HA, HB = slice(0, H), slice(H, C)

        def emit_add(t):
            u = up.tile([P, C], f32, tag="u", name=f"u{t}")
            u_t.append(u)
            if t in POOL_ADD:
                nc.gpsimd.tensor_add(u, x_sb[t], obc_sb)
            else:
                for h, hsl in enumerate((HA, HB)):
                    nc.vector.tensor_add(u[:, hsl], x_sb[t][:, hsl], obc_ps[h])

        def emit_sq(t):
            u = u_t[t]
            usq = sq.tile([P, C], f32, tag="usq", name=f"usq{t}")
            usq_t.append(usq)
            if t in SPLIT_SQ:
                # half A on ACT, half B on DVE; combine B's sum into the bias
                nc.scalar.activation(
                    usq[:, HA], u[:, HA], AF.Square,
                    accum_out=ssq_all[:, t, 0:1],
                )
                nc.vector.affine_mul_reduce(
                    usq[:, HB], ssq_all[:, t, 1:2], u[:, HB], u[:, HB], 1.0, 0.0
                )
                nc.vector.tensor_scalar(
                    emc_all[:, t : t + 1], ssq_all[:, t, 1:2],
                    1.0 / C, em_all[:, t : t + 1], op0=OP.mult, op1=OP.add,
                )
            elif t in DVE_SQ:
                nc.vector.affine_mul_reduce(
                    usq, ssq_all[:, t, 0:1], u, u, 1.0, 0.0
                )
            else:
                nc.scalar.activation(usq, u, AF.Square, accum_out=ssq_all[:, t, 0:1])

        def emit_rstd(t):
            bias = emc_all[:, t : t + 1] if t in SPLIT_SQ else em_all[:, t : t + 1]
            nc.scalar.activation(
                std_all[:, t : t + 1], ssq_all[:, t, 0:1], AF.Sqrt,
                bias=bias, scale=1.0 / C,
            )
            nc.vector.reciprocal(rstd_all[:, t : t + 1], std_all[:, t : t + 1])

        def emit_fin(t, sl):
            u = u_t[t]
            nc.vector.tensor_scalar(
                u[:, sl], u[:, sl], mu_all[:, t : t + 1],
                rstd_all[:, t : t + 1], op0=OP.subtract, op1=OP.mult,
            )
            nc.sync.dma_start(out=y[t * P : (t + 1) * P, sl], in_=u[:, sl])

        # explicit interleaving: adds 0-2 early on DVE, Pool runs 3-6,
        # add 7 squeezed onto DVE between fins, squares/rstds pipelined.
        for t in (0, 1, 2, 3, 4, 5, 6):
            emit_add(t)
        for t in (0, 1, 2):
            emit_sq(t)
        for t in (0, 1, 2):
            emit_rstd(t)
        for t in (0, 1, 2):
            emit_fin(t, HA)
            emit_fin(t, HB)
        for t in (3, 4, 5):
            emit_sq(t)
            emit_rstd(t)
            emit_fin(t, slice(0, C))
        emit_add(7)
        emit_sq(6)
        emit_rstd(6)
        emit_fin(6, slice(0, C))
        emit_sq(7)
        emit_rstd(7)
        emit_fin(7, slice(0, C))

    nc.compile()
    return nc


def _build(fast: bool):
    from contextlib import ExitStack

    import concourse.bacc as bacc_mod
    import concourse.mybir as mybir
    import concourse.tile as tile

    f32 = mybir.dt.float32
    bf16 = mybir.dt.bfloat16
    fp8 = mybir.dt.float8e4
    use_fp8 = FP8 and fast
    wdt = fp8 if use_fp8 else bf16
    s_gc = (1.0 / N) * (SG if use_fp8 else 1.0)
    s_vb = 1.0 / (SG * SW) if use_fp8 else 1.0
    s_h1 = SH / (SG * SW) if use_fp8 else 1.0
    s_ca = 1.0 / (SH * SW) if use_fp8 else 1.0
    s_g = SGT if use_fp8 else 1.0
    s_o = 1.0 / (SGT * SW) if use_fp8 else 1.0
    AF = mybir.ActivationFunctionType
    OP = mybir.AluOpType
    AX = mybir.AxisListType

    nc = bacc_mod.Bacc("TRN2")
    x = nc.dram_tensor("x", [N, C], f32, kind="ExternalInput")
    wv = nc.dram_tensor("wv", [C, C], wdt, kind="ExternalInput")
    w1 = nc.dram_tensor("w1", [C, CR], wdt, kind="ExternalInput")
    w2 = nc.dram_tensor("w2", [CR, C], wdt, kind="ExternalInput")
    wo = nc.dram_tensor("wo", [C, C], wdt, kind="ExternalInput")
    if not fast:
        # rows: 0=bv 1=b1(padded) 2=b2 3=bo 4=gamma 5=beta
        vecs = nc.dram_tensor("vecs", [1, 6, C], f32, kind="ExternalInput")
    y = nc.dram_tensor("y", [N, C], f32, kind="ExternalOutput")

    with tile.TileContext(nc) as tc, ExitStack() as ctx:
        xp = ctx.enter_context(tc.tile_pool(name="xp", bufs=NT))
        wp = ctx.enter_context(tc.tile_pool(name="wp", bufs=1))
        sm = ctx.enter_context(tc.tile_pool(name="sm", bufs=1))
        up = ctx.enter_context(tc.tile_pool(name="up", bufs=8))
        sq = ctx.enter_context(tc.tile_pool(name="sq", bufs=3))
        st = ctx.enter_context(tc.tile_pool(name="st", bufs=3))
        s8 = ctx.enter_context(tc.tile_pool(name="s8", bufs=NT))
        pp = ctx.enter_context(tc.tile_pool(name="pp", bufs=4, space="PSUM"))
        pc = ctx.enter_context(tc.tile_pool(name="pc", bufs=2, space="PSUM"))
        po = ctx.enter_context(tc.tile_pool(name="po", bufs=2, space="PSUM"))

        # ---- constants ----
        ones_col = sm.tile([P, 1], bf16)
        nc.vector.memset(ones_col, 1.0)
        ones_row = sm.tile([1, P], bf16 if fast else f32)
        nc.vector.memset(ones_row, 1.0)
        one11 = sm.tile([1, 1], wdt)
        nc.vector.memset(one11, 1.0)
        eps_t = sm.tile([P, 1], f32)
        nc.vector.memset(eps_t, LN_EPS)
        # warm the sigmoid activation table early (overlaps with DMA)
        dmy = sm.tile([1, 1], f32)
        nc.vector.memset(dmy, 0.0)
        dmy2 = sm.tile([1, 1], f32)
        nc.scalar.activation(dmy2, dmy, AF.Sigmoid)

        # ---- input DMAs: x first on SP FIFO, then weights in use order ----
        x_sb = []
        for t in range(NT):
            xt = xp.tile([P, C], f32, tag="x_sb", name=f"x_sb{t}")
            nc.sync.dma_start(out=xt, in_=x[t * P : (t + 1) * P, :])
            x_sb.append(xt)
        w1_sb = wp.tile([P, KC, CR], wdt)
        nc.sync.dma_start(out=w1_sb, in_=w1.rearrange("(a p) c -> p a c", p=P))
        w2_sb = wp.tile([P, 2, C], wdt)
        nc.sync.dma_start(out=w2_sb[:, 0, :], in_=w2[0:P, :])
        nc.sync.dma_start(out=w2_sb[0:64, 1, :], in_=w2[P:CR, :])
        wv_sb = wp.tile([P, KC, C], wdt)
        nc.sync.dma_start(out=wv_sb, in_=wv.rearrange("(a p) c -> p a c", p=P))
        wo_sb = wp.tile([P, KC, C], wdt)
        nc.sync.dma_start(out=wo_sb, in_=wo.rearrange("(a p) c -> p a c", p=P))
        if not fast:
            vec_sb = sm.tile([1, 6, C], f32)
            nc.sync.dma_start(out=vec_sb, in_=vecs[:, :, :])

        # ---- per-tile: bf16 cast (ACT) for column sums; row sums (DVE) ----
        xb_sb = []
        xsum_all = sm.tile([P, NT], f32, name="xsum_all") if fast else None
        for t in range(NT):
            xb = xp.tile([P, C], bf16, tag="xb_sb", name=f"xb_sb{t}")
            if fast:
                if t == NT - 1:
                    # last tile gates gc: cast on DVE (2x mode, faster)
                    nc.vector.tensor_scalar(
                        xb, x_sb[t], 1.0, 0.0, op0=OP.mult, op1=OP.add,
                        accum_out=xsum_all[:, t : t + 1],
                    )
                else:
                    nc.scalar.activation(
                        xb, x_sb[t], AF.Copy, accum_out=xsum_all[:, t : t + 1]
                    )
            else:
                nc.scalar.copy(xb, x_sb[t])
            xb_sb.append(xb)

        # ---- gc = mean_n x  (column sums via PE; ones stationary) ----
        cs_ps = [pp.tile([1, H], f32, tag="rowps", name=f"cs_ps{h}") for h in range(NH)]
        for t in range(NT):
            for h in range(NH):
                nc.tensor.matmul(
                    cs_ps[h],
                    ones_col,
                    xb_sb[t][:, h * H : (h + 1) * H],
                    start=(t == 0),
                    stop=(t == NT - 1),
                )
        gc_sb = sm.tile([1, C], wdt)
        nc.scalar.activation(gc_sb[0:1, 0:H], cs_ps[0], AF.Copy, bias=0.0, scale=s_gc)
        nc.vector.tensor_scalar_mul(gc_sb[0:1, H : 2 * H], cs_ps[1], s_gc)

        # ---- transpose gc to partitions: gcT[:, j] = gc[128j:128j+128] ----
        gcT_ps = pc.tile([P, 8], f32, tag="colps")
        for j in range(KC):
            nc.tensor.matmul(
                gcT_ps[:, j : j + 1], gc_sb[0:1, j * P : (j + 1) * P], one11,
                start=True, stop=True,
            )
        gcT_sb = sm.tile([P, KC], wdt)
        nc.vector.tensor_copy(gcT_sb, gcT_ps[:, 0:KC])

        # ---- h1 = relu(gc @ W1 (+ b1)) ----
        h1_ps = pp.tile([1, CR], f32, tag="rowps")
        for j in range(KC):
            nc.tensor.matmul(
                h1_ps, gcT_sb[:, j : j + 1], w1_sb[:, j, :],
                start=(j == 0), stop=(j == KC - 1),
            )
        vb_ps = [pp.tile([1, H], f32, tag="rowps", name=f"vb_ps{h}") for h in range(NH)]
        for j in range(KC):
            nc.tensor.matmul(
                vb_ps[0],
                gcT_sb[:, j : j + 1],
                wv_sb[:, j, 0:H],
                start=(j == 0),
                stop=(j == KC - 1),
            )
        h1r = sm.tile([1, CR], wdt)
        if fast:
            nc.scalar.activation(h1r, h1_ps, AF.Relu, scale=s_h1)
        else:
            h1f = sm.tile([1, CR], f32)
            nc.vector.tensor_add(h1f, h1_ps, vec_sb[0:1, 1, 0:CR])
            nc.vector.tensor_scalar_max(h1r, h1f, 0.0)

        # ---- transpose h1 (192 = 128 + 64) ----
        h1T_ps = pc.tile([P, 8], f32, tag="colps")
        nc.tensor.matmul(h1T_ps[:, 0:1], h1r[0:1, 0:P], one11, start=True, stop=True)
        nc.tensor.matmul(
            h1T_ps[0:64, 1:2], h1r[0:1, P:CR], one11, start=True, stop=True
        )
        h1T_sb = sm.tile([P, 2], wdt)
        nc.vector.tensor_copy(h1T_sb[:, 0:1], h1T_ps[:, 0:1])
        nc.vector.tensor_copy(h1T_sb[0:64, 1:2], h1T_ps[0:64, 1:2])

        # ---- ca = sigmoid(h1 @ W2 (+ b2)) ----
        ca_ps = [pp.tile([1, H], f32, tag="rowps", name=f"ca_ps{h}") for h in range(NH)]
        for h in range(NH):
            sl = slice(h * H, (h + 1) * H)
            nc.tensor.matmul(
                ca_ps[h], h1T_sb[:, 0:1], w2_sb[:, 0, sl], start=True, stop=False
            )
            nc.tensor.matmul(
                ca_ps[h], h1T_sb[0:64, 1:2], w2_sb[0:64, 1, sl],
                start=False, stop=True,
            )
        ca_sb = sm.tile([1, C], f32)
        if fast:
            for h in range(NH):
                sl = slice(h * H, (h + 1) * H)
                nc.scalar.activation(ca_sb[0:1, sl], ca_ps[h], AF.Sigmoid, scale=s_ca)
        else:
            cap_f = sm.tile([1, C], f32)
            for h in range(NH):
                sl = slice(h * H, (h + 1) * H)
                nc.vector.tensor_add(cap_f[0:1, sl], ca_ps[h], vec_sb[0:1, 2, sl])
            nc.scalar.activation(ca_sb, cap_f, AF.Sigmoid)

        # ---- vbar = gc @ Wv (+ bv) ----
        vbar_sb = sm.tile([1, C], f32)
        if fast:
            nc.scalar.activation(
                vbar_sb[0:1, 0:H], vb_ps[0], AF.Copy, bias=0.0, scale=s_vb
            )
        else:
            nc.vector.tensor_add(vbar_sb[0:1, 0:H], vb_ps[0], vec_sb[0:1, 0, 0:H])

        for j in range(KC):
            nc.tensor.matmul(
                vb_ps[1],
                gcT_sb[:, j : j + 1],
                wv_sb[:, j, H : 2 * H],
                start=(j == 0),
                stop=(j == KC - 1),
            )
        if fast:
            nc.vector.tensor_scalar_mul(vbar_sb[0:1, H : 2 * H], vb_ps[1], s_vb)
        else:
            nc.vector.tensor_add(
                vbar_sb[0:1, H : 2 * H], vb_ps[1], vec_sb[0:1, 0, H : 2 * H]
            )

        # ---- g = vbar * ca ; transpose ; o = g @ Wo — in halves ----
        g_sb = sm.tile([1, C], wdt)
        gT_ps = pc.tile([P, 8], f32, tag="colps")
        gT_sb = sm.tile([P, KC], wdt)
        o_ps = [pp.tile([1, H], f32, tag="rowps", name=f"o_ps{h}") for h in range(NH)]
        for half in range(2):
            hs = slice(half * H, (half + 1) * H)
            if use_fp8:
                nc.vector.scalar_tensor_tensor(
                    out=g_sb[0:1, hs], in0=vbar_sb[0:1, hs], scalar=s_g,
                    in1=ca_sb[0:1, hs], op0=OP.mult, op1=OP.mult,
                )
            else:
                nc.vector.tensor_mul(
                    g_sb[0:1, hs], vbar_sb[0:1, hs], ca_sb[0:1, hs]
                )
            for j in range(half * 3, half * 3 + 3):
                nc.tensor.matmul(
                    gT_ps[:, j : j + 1], g_sb[0:1, j * P : (j + 1) * P], one11,
                    start=True, stop=True,
                )
            nc.vector.tensor_copy(
                gT_sb[:, half * 3 : half * 3 + 3],
                gT_ps[:, half * 3 : half * 3 + 3],
            )
            for j in range(half * 3, half * 3 + 3):
                for h in range(NH):
                    nc.tensor.matmul(
                        o_ps[h],
                        gT_sb[:, j : j + 1],
                        wo_sb[:, j, h * H : (h + 1) * H],
                        start=(j == 0),
                        stop=(j == KC - 1),
                    )

        o_sb = sm.tile([1, C], bf16 if fast else f32)
        if fast:
            nc.scalar.activation(o_sb[0:1, 0:H], o_ps[0], AF.Copy, bias=0.0, scale=s_o)
            nc.vector.tensor_scalar_mul(o_sb[0:1, H : 2 * H], o_ps[1], s_o)
        else:
            for h in range(NH):
                sl = slice(h * H, (h + 1) * H)
                nc.vector.tensor_add(o_sb[0:1, sl], o_ps[h], vec_sb[0:1, 3, sl])

        # ---- broadcast o across partitions via K=1 matmul ----
        obc_sb = sm.tile([P, C], f32)
        for h in range(NH):
            sl = slice(h * H, (h + 1) * H)
            obp = po.tile([P, H], f32, tag="obc", name=f"obp{h}")
            nc.tensor.matmul(obp, ones_row, o_sb[0:1, sl], start=True, stop=True)
            nc.scalar.copy(obc_sb[:, sl], obp)

        if fast:
            # sum(o) on one partition, then broadcast to (128,1) via K=1 matmul
            osum_row = sm.tile([1, 1], f32)
            nc.vector.tensor_reduce(osum_row, o_sb, AX.X, OP.add)
            osum_row_b = sm.tile([1, 1], bf16)
            nc.vector.tensor_copy(osum_row_b, osum_row)
            osb_ps = po.tile([P, 1], f32, tag="obc")
            nc.tensor.matmul(osb_ps, ones_row, osum_row_b, start=True, stop=True)
            osum = sm.tile([P, 1], f32)
            nc.vector.tensor_copy(osum, osb_ps)

            # batched per-tile stats: mu, mu^2, (eps - mu^2)
            mu_all = sm.tile([P, NT], f32)
            nc.vector.tensor_scalar(
                mu_all, xsum_all, osum, 1.0 / C, op0=OP.add, op1=OP.mult
            )
            musq_all = sm.tile([P, NT], f32)
            nc.vector.tensor_mul(musq_all, mu_all, mu_all)
            em_all = sm.tile([P, NT], f32)
            nc.vector.tensor_scalar(
                em_all, musq_all, -1.0, LN_EPS, op0=OP.mult, op1=OP.add
            )

            for t in range(NT):
                on_dve = t >= NT - N_DVE_TILES
                u = up.tile([P, C], f32, tag="u", name=f"u{t}")
                # u = x + o on DVE or Pool; mu folded into the final scale op
                if on_dve:
                    nc.vector.scalar_tensor_tensor(
                        out=u, in0=x_sb[t], scalar=0.0, in1=obc_sb,
                        op0=OP.bypass, op1=OP.add,
                    )
                else:
                    nc.gpsimd.tensor_add(u, x_sb[t], obc_sb)
                usq = sq.tile([P, C], f32, tag="usq")
                uss = s8.tile([P, 1], f32, tag="uss", name=f"uss{t}")
                nc.scalar.activation(usq, u, AF.Square, accum_out=uss)
                # std = sqrt(uss/C + eps - mu^2)
                std = st.tile([P, 1], f32, tag="std")
                nc.scalar.activation(
                    std, uss, AF.Sqrt, bias=em_all[:, t : t + 1], scale=1.0 / C
                )
                rstd = st.tile([P, 1], f32, tag="rstd")
                nc.vector.reciprocal(rstd, std)
                nc.vector.tensor_scalar(
                    u, u, mu_all[:, t : t + 1], rstd, op0=OP.subtract, op1=OP.mult
                )
                nc.scalar.dma_start(out=y[t * P : (t + 1) * P, :], in_=u)
        else:
            gamma_bc = sm.tile([P, C], f32)
            beta_bc = sm.tile([P, C], f32)
            for h in range(NH):
                sl = slice(h * H, (h + 1) * H)
                gbp = po.tile([P, H], f32, tag="obc", name=f"gbp{h}")
                nc.tensor.matmul(
                    gbp, ones_row, vec_sb[0:1, 4, sl], start=True, stop=True
                )
                nc.vector.tensor_copy(gamma_bc[:, sl], gbp)
            for h in range(NH):
                sl = slice(h * H, (h + 1) * H)
                bbp = po.tile([P, H], f32, tag="obc", name=f"bbp{h}")
                nc.tensor.matmul(
                    bbp, ones_row, vec_sb[0:1, 5, sl], start=True, stop=True
                )
                nc.vector.tensor_copy(beta_bc[:, sl], bbp)

            for t in range(NT):
                z = x_sb[t]
                nc.gpsimd.tensor_add(z, z, obc_sb)
                stats = sq.tile([P, 3, 6], f32, tag="stats", name=f"stats{t}")
                zg = z.rearrange("p (s d) -> p s d", s=3)
                for s in range(3):
                    nc.vector.bn_stats(stats[:, s, :], zg[:, s, :])
                mv = st.tile([P, 2], f32, tag="mv")
                nc.vector.bn_aggr(mv, stats)
                std = st.tile([P, 1], f32, tag="std")
                nc.scalar.activation(std, mv[:, 1:2], AF.Sqrt, bias=eps_t)
                rstd = st.tile([P, 1], f32, tag="rstd")
                nc.vector.reciprocal(rstd, std)
                zq = up.tile([P, C], f32, tag="u")
                nc.vector.scalar_tensor_tensor(
                    out=zq, in0=z, scalar=mv[:, 0:1], in1=gamma_bc,
                    op0=OP.subtract, op1=OP.mult,
                )
                nc.vector.tensor_scalar_mul(zq, zq, rstd)
                nc.vector.tensor_add(zq, zq, beta_bc)
                nc.scalar.dma_start(out=y[t * P : (t + 1) * P, :], in_=zq)

    nc.compile()
    return nc




def _build_general():
    return _build(False)


def _get_nc(fast: bool):
    key = ("nc", fast)
    if key not in _CACHE:
        _CACHE[key] = _build_fast() if fast else _build_general()
    return _CACHE[key]


def _pack(w, rows, wdt, scale):
    """Pack a (rows, cols) weight as [128, ceil(rows/128), cols] fp8/bf16."""
    a = -(-rows // P)
    out = np.zeros((P, a, w.shape[1]), np.float32)
    wf = np.asarray(w, np.float32) * scale
    for j in range(a):
        r = wf[j * P : (j + 1) * P]
        out[: r.shape[0], j] = r
    return np.ascontiguousarray(out.astype(wdt))


def make_in_maps(x, Wv, bv, W1, b1, W2, b2, Wo, bo, gamma, beta, fast=True):
    if FP8 and fast:
        import concourse.mybir as mybir

        wdt = mybir.dt.np(mybir.dt.float8e4)
        s = SW
    else:
        wdt = ml_dtypes.bfloat16
        s = 1.0
    if fast:
        ident = np.eye(P, dtype=ml_dtypes.bfloat16)
        shared = {
            "wv": _pack(Wv, C, wdt, s),
            "w1": _pack(W1, C, wdt, s),
            "w2": _pack(W2, CR, wdt, s),
            "wo": _pack(Wo, C, wdt, s),
            "ident": np.ascontiguousarray(ident),
        }
    else:
        shared = {
            "wv": np.ascontiguousarray(np.asarray(Wv, np.float32).astype(wdt)),
            "w1": np.ascontiguousarray(np.asarray(W1, np.float32).astype(wdt)),
            "w2": np.ascontiguousarray(np.asarray(W2, np.float32).astype(wdt)),
            "wo": np.ascontiguousarray(np.asarray(Wo, np.float32).astype(wdt)),
        }
        b1p = np.zeros(C, np.float32)
        b1p[:CR] = np.asarray(b1, np.float32)
        vecs = np.stack(
            [
                np.asarray(bv, np.float32),
                b1p,
                np.asarray(b2, np.float32),
                np.asarray(bo, np.float32),
                np.asarray(gamma, np.float32),
                np.asarray(beta, np.float32),
            ]
        )
        shared["vecs"] = np.ascontiguousarray(vecs.reshape(1, 6, C))
    return [
        {"x": np.ascontiguousarray(np.asarray(x[i], np.float32)), **shared}
        for i in range(NCORES)
    ]


def _is_fast(inputs):
    def z(a):
        return bool(np.all(np.asarray(a) == 0.0))

    return (
        bool(np.all(np.asarray(inputs["gamma"]) == 1.0))
        and z(inputs["beta"]) and z(inputs["bv"]) and z(inputs["b1"])
        and z(inputs["b2"]) and z(inputs["bo"])
    )


def run(inputs, trace=False, **kw):
    from concourse.bass_utils import run_bass_kernel_spmd

    fast = _is_fast(inputs)
    nc = _get_nc(fast)
    in_maps = make_in_maps(
        inputs["x"], inputs["Wv"], inputs["bv"], inputs["W1"], inputs["b1"],
        inputs["W2"], inputs["b2"], inputs["Wo"], inputs["bo"],
        inputs["gamma"], inputs["beta"], fast=fast,
    )
    res = run_bass_kernel_spmd(nc, in_maps, list(range(NCORES)), trace=trace, **kw)
    out = np.stack([r["y"] for r in res.results]).astype(np.float32)
    return out, res


def kernel(
    x, Wq, bq, Wk, bk, Wv, bv, W1, b1, W2, b2, Wo, bo, gamma, beta
) -> np.ndarray:
    # Wq/bq/Wk/bk provably do not affect the output (uniform softmax).
    out, _ = run(
        dict(
            x=x, Wv=Wv, bv=bv, W1=W1, b1=b1, W2=W2, b2=b2, Wo=Wo, bo=bo,
            gamma=gamma, beta=beta,
        )
    )
    return out


# revision 23
# speedup vs baseline: 1.2170x; 1.0794x over previous
"""GCAttention Trainium2 kernel.

Math: in the reference, k = broadcast(gc @ Wk + bk) has identical rows, so
attention scores are constant along the softmax axis -> softmax is exactly
uniform -> attn @ v = mean_n(v) = gc @ Wv + bv (broadcast over tokens).
The whole module therefore reduces to:
    gc   = mean_n x                       (B, C)
    vbar = gc @ Wv + bv                   (B, C)
    ca   = sigmoid(relu(gc@W1+b1)@W2+b2)  (B, C)
    o    = (vbar * ca) @ Wo + bo          (B, C)
    y    = LN(x + o[:,None,:]) * gamma + beta
Sharding: data-parallel over batch B=8 across the 8 cores (1 batch each).

Fast path: all GEMVs are column-oriented on the PE (outputs land as PSUM
columns [128,1] per chunk), the column sums for gc are f32r matmuls with the
x tiles as stationary, and the LN phase overlaps the y writeback DMAs with
adds on DVE/Pool, squares on ACT/DVE, rstd on ACT (Rsqrt), finals on DVE.
"""

import numpy as np
import ml_dtypes

B, N, C = 8, 1024, 768
CR = 192
P = 128
NT = N // P   # 8 token tiles per core
KC = C // P   # 6 channel chunks
H = 384
LN_EPS = 1e-5
NCORES = 8
FP8 = True
SW = 16.0  # host weight scale
SG = 16.0  # gc scale
SH = 16.0  # h1 scale
SGT = 32.0  # g scale

_CACHE: dict = {}


def _build_fast():
    from contextlib import ExitStack

    import concourse.bacc as bacc_mod
    import concourse.mybir as mybir
    import concourse.tile as tile

    f32 = mybir.dt.float32
    f32r = mybir.dt.float32r
    bf16 = mybir.dt.bfloat16
    fp8 = mybir.dt.float8e4
    wdt = fp8 if FP8 else bf16
    s_gc = (1.0 / N) * (SG if FP8 else 1.0)
    s_h1 = SH / (SG * SW) if FP8 else 1.0
    s_ca = 1.0 / (SH * SW) if FP8 else 1.0
    s_vb = 1.0 / (SG * SW) if FP8 else 1.0
    s_g = SGT if FP8 else 1.0
    s_o = 1.0 / (SGT * SW) if FP8 else 1.0
    AF = mybir.ActivationFunctionType
    OP = mybir.AluOpType
    AX = mybir.AxisListType

    nc = bacc_mod.Bacc("TRN2")
    x = nc.dram_tensor("x", [N, C], f32, kind="ExternalInput")
    # blob: w1 [P, KC*CR] fp8 ++ w2 [P, 2*C] fp8 ++ ident [P, P] bf16(2B)
    BLOB = KC * CR + 2 * C + 2 * P
    blob = nc.dram_tensor("blob", [P, BLOB], wdt, kind="ExternalInput")
    wv = nc.dram_tensor("wv", [P, KC, C], wdt, kind="ExternalInput")
    wo_a = nc.dram_tensor("wo_a", [P, KC, H], wdt, kind="ExternalInput")
    wo_b = nc.dram_tensor("wo_b", [P, KC, H], wdt, kind="ExternalInput")
    y = nc.dram_tensor("y", [N, C], f32, kind="ExternalOutput")

    # LN engine assignment per tile. Adds for 0-2 and 7 on DVE (reading obc
    # straight from PSUM, in halves), 3-6 on Pool (from the SBUF copy).
    # Squares: tiles 0/1 split ACT||DVE to shorten the first writeback's
    # latency; 2-5 whole on ACT; 6/7 whole on DVE (AFFINE_MUL_REDUCE).
    POOL_ADD = {3, 4, 5, 6}
    SPLIT_SQ = {0, 1}
    DVE_SQ = {6, 7}

    with tile.TileContext(nc) as tc, ExitStack() as ctx:
        xp = ctx.enter_context(tc.tile_pool(name="xp", bufs=NT))
        wp = ctx.enter_context(tc.tile_pool(name="wp", bufs=1))
        sm = ctx.enter_context(tc.tile_pool(name="sm", bufs=1))
        up = ctx.enter_context(tc.tile_pool(name="up", bufs=NT))
        sq = ctx.enter_context(tc.tile_pool(name="sq", bufs=3))
        # PSUM: pc = chain columns (1 bank), pr = o_row (1), pb = obc (2)
        pc = ctx.enter_context(tc.tile_pool(name="pc", bufs=1, space="PSUM"))
        pr = ctx.enter_context(tc.tile_pool(name="pr", bufs=1, space="PSUM"))
        pb = ctx.enter_context(tc.tile_pool(name="pb", bufs=1, space="PSUM"))

        # ---- constants ----
        ones_colr = sm.tile([P, 1], f32)
        nc.vector.memset(ones_colr, 1.0)
        ones_colb = sm.tile([P, 1], bf16)
        nc.vector.memset(ones_colb, 1.0)
        ones_rowb = sm.tile([1, P], bf16)
        nc.vector.memset(ones_rowb, 1.0)
        # warm the sigmoid table early (set also holds relu/copy/square)
        dmy = sm.tile([1, 1], f32)
        nc.vector.memset(dmy, 1.0)
        dmy2 = sm.tile([1, 1], f32)
        nc.scalar.activation(dmy2, dmy, AF.Sigmoid)

        # ---- input DMAs: x first, then weights in chain-use order ----
        x_sb = []
        for t in range(NT):
            xt = xp.tile([P, C], f32, tag="x_sb", name=f"x_sb{t}")
            nc.sync.dma_start(out=xt, in_=x[t * P : (t + 1) * P, :])
            x_sb.append(xt)
        blob_sb = wp.tile([P, BLOB], wdt)
        nc.sync.dma_start(out=blob_sb, in_=blob[:, :])
        w1_sb = blob_sb[:, 0 : KC * CR].rearrange("p (a c) -> p a c", a=KC)
        w2_sb = blob_sb[:, KC * CR : KC * CR + 2 * C].rearrange(
            "p (a c) -> p a c", a=2
        )
        id_sb = blob_sb[:, KC * CR + 2 * C :].bitcast(bf16)
        wv_sb = wp.tile([P, KC, C], wdt)
        nc.sync.dma_start(out=wv_sb, in_=wv[:, :, :])
        wo_sb = []
        for hh in (wo_a, wo_b):
            wt = wp.tile([P, KC, H], wdt, name=f"wo_sb{hh.name}")
            nc.sync.dma_start(out=wt, in_=hh[:, :, :])
            wo_sb.append(wt)

        # ---- per tile: rowsums (DVE) + transposed colsums (PE, f32r) ----
        xsum_all = sm.tile([P, NT], f32)
        gcT_ps = pc.tile([P, 8], f32, tag="colA")
        xb_sb = []
        for t in range(NT):
            xb = xp.tile([P, C], bf16, tag="xb_sb", name=f"xb_sb{t}")
            nc.vector.tensor_scalar(
                xb, x_sb[t], 1.0, 0.0, op0=OP.mult, op1=OP.add,
                accum_out=xsum_all[:, t : t + 1],
            )
            xb_sb.append(xb)
        for j in range(KC):
            for t in range(NT):
                nc.tensor.matmul(
                    gcT_ps[:, j : j + 1],
                    x_sb[t][:, j * P : (j + 1) * P],
                    ones_colr,
                    start=(t == 0),
                    stop=(t == NT - 1),
                )

        # ---- gcT -> SBUF fp8 (scaled) ----
        gcT_sb = sm.tile([P, KC], wdt)
        nc.vector.tensor_scalar_mul(gcT_sb, gcT_ps[:, 0:KC], s_gc)

        # ---- chain columns tile: h1T(0:2) caT(2:8) vbarT(8:14) oT(14:20) ----
        ch_ps = pc.tile([P, 20], f32, tag="colB")

        # h1T = W1.T @ gc  (m-chunks: 128 + 64)
        for j in range(KC):
            nc.tensor.matmul(
                ch_ps[:, 0:1], w1_sb[:, j, 0:P], gcT_sb[:, j : j + 1],
                start=(j == 0), stop=(j == KC - 1),
            )
        for j in range(KC):
            nc.tensor.matmul(
                ch_ps[0:64, 1:2], w1_sb[:, j, P:CR], gcT_sb[:, j : j + 1],
                start=(j == 0), stop=(j == KC - 1),
            )
        # relu + scale -> fp8
        h1T_sb = sm.tile([P, 2], wdt)
        nc.vector.tensor_scalar(
            h1T_sb[:, 0:1], ch_ps[:, 0:1], s_h1, 0.0, op0=OP.mult, op1=OP.max
        )
        nc.vector.tensor_scalar(
            h1T_sb[0:64, 1:2], ch_ps[0:64, 1:2], s_h1, 0.0, op0=OP.mult, op1=OP.max
        )

        # caT = W2.T @ h1 (k: 128 + 64)
        for m in range(KC):
            nc.tensor.matmul(
                ch_ps[:, 2 + m : 3 + m],
                w2_sb[:, 0, m * P : (m + 1) * P],
                h1T_sb[:, 0:1],
                start=True, stop=False,
            )
            nc.tensor.matmul(
                ch_ps[:, 2 + m : 3 + m],
                w2_sb[0:64, 1, m * P : (m + 1) * P],
                h1T_sb[0:64, 1:2],
                start=False, stop=True,
            )
        caT_sb = sm.tile([P, KC], f32)
        nc.scalar.activation(caT_sb, ch_ps[:, 2:8], AF.Sigmoid, scale=s_ca)
        # force the ACT table switch (sigmoid set -> sqrt/square set) to
        # happen right after the sigmoid, inside the wv/wo DMA-wait window;
        # reading caT_sb pins this after the sigmoid in the schedule
        dmy3 = sm.tile([1, 1], f32)
        nc.scalar.activation(dmy3, caT_sb[0:1, 0:1], AF.Sqrt)

        # vbarT = Wv.T @ gc
        for m in range(KC):
            for j in range(KC):
                nc.tensor.matmul(
                    ch_ps[:, 8 + m : 9 + m],
                    wv_sb[:, j, m * P : (m + 1) * P],
                    gcT_sb[:, j : j + 1],
                    start=(j == 0), stop=(j == KC - 1),
                )
        # gT = (vbar * s) * ca -> fp8
        gT_sb = sm.tile([P, KC], wdt)
        nc.vector.scalar_tensor_tensor(
            out=gT_sb, in0=ch_ps[:, 8:14], scalar=s_g * s_vb, in1=caT_sb,
            op0=OP.mult, op1=OP.mult,
        )

        # oT = Wo.T @ g, transpose, broadcast -- pipelined per wo half
        oT_sb = sm.tile([P, KC], bf16)
        orow_ps = pr.tile([1, C], bf16, tag="orow", name="orow_ps")
        orow_sb = sm.tile([1, C], bf16)
        obc_sb = sm.tile([P, C], bf16)
        obc_ps = []
        for h in range(2):
            for m in range(3 * h, 3 * h + 3):
                for j in range(KC):
                    nc.tensor.matmul(
                        ch_ps[:, 14 + m : 15 + m],
                        wo_sb[h][:, j, (m - 3 * h) * P : (m - 3 * h + 1) * P],
                        gT_sb[:, j : j + 1],
                        start=(j == 0), stop=(j == KC - 1),
                    )
            hsl = slice(h * H, (h + 1) * H)
            nc.vector.tensor_scalar_mul(
                oT_sb[:, 3 * h : 3 * h + 3], ch_ps[:, 14 + 3 * h : 17 + 3 * h], s_o
            )
            for m in range(3 * h, 3 * h + 3):
                nc.tensor.transpose(
                    orow_ps[0:1, m * P : (m + 1) * P], oT_sb[:, m : m + 1], id_sb
                )
            nc.vector.tensor_copy(orow_sb[0:1, hsl], orow_ps[0:1, hsl])
            ob = pb.tile([P, H], f32, tag=f"obc{h}", name=f"obc_ps{h}")
            for m in range(3):
                nc.tensor.matmul(
                    ob[:, m * P : (m + 1) * P],
                    ones_rowb,
                    orow_sb[0:1, (3 * h + m) * P : (3 * h + m + 1) * P],
                    start=True, stop=True,
                )
            obc_ps.append(ob)
            # bf16 SBUF copy (Pool + bf16 DVE adds read this)
            nc.scalar.copy(obc_sb[:, hsl], ob)

        # ---- osum = sum(o): column-sum oT then row-reduce ----
        os_ps = pr.tile([1, 8], f32, tag="osps", name="os_ps")
        nc.tensor.matmul(os_ps[0:1, 0:KC], ones_colb, oT_sb, start=True, stop=True)
        osum_row = sm.tile([1, 1], f32)
        nc.vector.tensor_reduce(osum_row, os_ps[:, 0:KC], AX.X, OP.add)
        osum_rb = sm.tile([1, 1], bf16)
        nc.vector.tensor_copy(osum_rb, osum_row)
        osb_ps = pr.tile([P, 1], f32, tag="osps", name="osb_ps")
        nc.tensor.matmul(osb_ps, ones_rowb, osum_rb, start=True, stop=True)
        osum = sm.tile([P, 1], f32)
        nc.vector.tensor_copy(osum, osb_ps)

        # batched stats: mu, eps - mu^2
        mu_all = sm.tile([P, NT], f32)
        nc.vector.tensor_scalar(
            mu_all, xsum_all, osum, 1.0 / C, op0=OP.add, op1=OP.mult
        )
        musq_all = sm.tile([P, NT], f32)
        nc.vector.tensor_mul(musq_all, mu_all, mu_all)
        em_all = sm.tile([P, NT], f32)
        nc.vector.tensor_scalar(
            em_all, musq_all, -1.0, LN_EPS, op0=OP.mult, op1=OP.add
        )

        # ---- LN phase ----
        # u dtype: tiles 0/1 f32 (adds from obc PSUM halves), 2-7 bf16
        # (adds from the bf16 SBUF copy; 2x DVE mode / Pool).
        ssq_all = sm.tile([P, NT, 2], f32)
        emc0 = sm.tile([P, 1], f32)
        std_all = sm.tile([P, NT], f32)
        rstd_all = sm.tile([P, NT], f32)
        HA, HB = slice(0, H), slice(H, C)
        u_t = {}

        def emit_add_f32(t):
            u = up.tile([P, C], f32, tag="u32", name=f"u{t}")
            u_t[t] = u
            for h, hsl in enumerate((HA, HB)):
                nc.vector.tensor_add(u[:, hsl], x_sb[t][:, hsl], obc_ps[h])

        def emit_add_b16(t, pool):
            u = up.tile([P, C], bf16, tag="u16", name=f"u{t}")
            u_t[t] = u
            if pool:
                nc.gpsimd.tensor_add(u, xb_sb[t], obc_sb)
            else:
                nc.vector.tensor_add(u, xb_sb[t], obc_sb)

        def emit_sq_act(t, sl=slice(0, C), s=0):
            u = u_t[t]
            usq = sq.tile([P, C], u.dtype, tag="usq", name=f"usq{t}")
            nc.scalar.activation(
                usq[:, sl], u[:, sl], AF.Square, accum_out=ssq_all[:, t, s : s + 1]
            )

        def emit_sq_amr(t, sl=slice(0, C), s=0):
            u = u_t[t]
            usq = sq.tile([P, C], u.dtype, tag="usq", name=f"usqd{t}")
            nc.vector.affine_mul_reduce(
                usq[:, sl], ssq_all[:, t, s : s + 1], u[:, sl], u[:, sl], 1.0, 0.0
            )

        def emit_std(t, bias):
            nc.scalar.activation(
                std_all[:, t : t + 1], ssq_all[:, t, 0:1], AF.Sqrt,
                bias=bias, scale=1.0 / C,
            )

        def emit_fin(t, sl):
            u = u_t[t]
            nc.vector.reciprocal(rstd_all[:, t : t + 1], std_all[:, t : t + 1])
            yt = up.tile([P, C], f32, tag="yt", name=f"y{t}") if u.dtype != f32 else u
            nc.vector.tensor_scalar(
                yt[:, sl], u[:, sl], mu_all[:, t : t + 1],
                rstd_all[:, t : t + 1], op0=OP.subtract, op1=OP.mult,
            )
            nc.sync.dma_start(out=y[t * P : (t + 1) * P, sl], in_=yt[:, sl])

        def emit_fin2(t, sl):
            # second half: reciprocal already done
            u = u_t[t]
            yt = u
            nc.vector.tensor_scalar(
                yt[:, sl], u[:, sl], mu_all[:, t : t + 1],
                rstd_all[:, t : t + 1], op0=OP.subtract, op1=OP.mult,
            )
            nc.sync.dma_start(out=y[t * P : (t + 1) * P, sl], in_=yt[:, sl])

        # tile 0: split square ACT(A) || DVE(B), earliest writeback
        emit_add_f32(0)
        emit_sq_act(0, HA, 0)
        emit_sq_amr(0, HB, 1)
        nc.vector.tensor_scalar(
            emc0, ssq_all[:, 0, 1:2], 1.0 / C, em_all[:, 0:1],
            op0=OP.mult, op1=OP.add,
        )
        emit_std(0, emc0)
        emit_add_f32(1)
        emit_fin(0, HA)
        emit_fin2(0, HB)
        # tile 1
        emit_sq_act(1)
        emit_std(1, em_all[:, 1:2])
        emit_fin(1, slice(0, C))
        # Pool adds for 2-5 (emitted up-front so Pool starts at obc_sb)
        for t in (2, 3, 4, 5):
            emit_add_b16(t, pool=True)
        emit_add_b16(6, pool=False)
        for t in (2, 3, 4):
            emit_sq_act(t)
            emit_std(t, em_all[:, t : t + 1])
            emit_fin(t, slice(0, C))
        emit_add_b16(7, pool=False)
        emit_sq_act(5)
        emit_sq_amr(6)
        emit_std(5, em_all[:, 5:6])
        emit_fin(5, slice(0, C))
        emit_std(6, em_all[:, 6:7])
        emit_fin(6, slice(0, C))
        emit_sq_amr(7)
        emit_std(7, em_all[:, 7:8])
        emit_fin(7, slice(0, C))

    nc.compile()
    return nc


def _build(fast: bool):
    from contextlib import ExitStack

    import concourse.bacc as bacc_mod
    import concourse.mybir as mybir
    import concourse.tile as tile

    f32 = mybir.dt.float32
    bf16 = mybir.dt.bfloat16
    fp8 = mybir.dt.float8e4
    use_fp8 = FP8 and fast
    wdt = fp8 if use_fp8 else bf16
    s_gc = (1.0 / N) * (SG if use_fp8 else 1.0)
    s_vb = 1.0 / (SG * SW) if use_fp8 else 1.0
    s_h1 = SH / (SG * SW) if use_fp8 else 1.0
    s_ca = 1.0 / (SH * SW) if use_fp8 else 1.0
    s_g = SGT if use_fp8 else 1.0
    s_o = 1.0 / (SGT * SW) if use_fp8 else 1.0
    AF = mybir.ActivationFunctionType
    OP = mybir.AluOpType
    AX = mybir.AxisListType

    nc = bacc_mod.Bacc("TRN2")
    x = nc.dram_tensor("x", [N, C], f32, kind="ExternalInput")
    wv = nc.dram_tensor("wv", [C, C], wdt, kind="ExternalInput")
    w1 = nc.dram_tensor("w1", [C, CR], wdt, kind="ExternalInput")
    w2 = nc.dram_tensor("w2", [CR, C], wdt, kind="ExternalInput")
    wo = nc.dram_tensor("wo", [C, C], wdt, kind="ExternalInput")
    if not fast:
        # rows: 0=bv 1=b1(padded) 2=b2 3=bo 4=gamma 5=beta
        vecs = nc.dram_tensor("vecs", [1, 6, C], f32, kind="ExternalInput")
    y = nc.dram_tensor("y", [N, C], f32, kind="ExternalOutput")

    with tile.TileContext(nc) as tc, ExitStack() as ctx:
        xp = ctx.enter_context(tc.tile_pool(name="xp", bufs=NT))
        wp = ctx.enter_context(tc.tile_pool(name="wp", bufs=1))
        sm = ctx.enter_context(tc.tile_pool(name="sm", bufs=1))
        up = ctx.enter_context(tc.tile_pool(name="up", bufs=8))
        sq = ctx.enter_context(tc.tile_pool(name="sq", bufs=3))
        st = ctx.enter_context(tc.tile_pool(name="st", bufs=3))
        s8 = ctx.enter_context(tc.tile_pool(name="s8", bufs=NT))
        pp = ctx.enter_context(tc.tile_pool(name="pp", bufs=4, space="PSUM"))
        pc = ctx.enter_context(tc.tile_pool(name="pc", bufs=2, space="PSUM"))
        po = ctx.enter_context(tc.tile_pool(name="po", bufs=2, space="PSUM"))

        # ---- constants ----
        ones_col = sm.tile([P, 1], bf16)
        nc.vector.memset(ones_col, 1.0)
        ones_row = sm.tile([1, P], bf16 if fast else f32)
        nc.vector.memset(ones_row, 1.0)
        one11 = sm.tile([1, 1], wdt)
        nc.vector.memset(one11, 1.0)
        eps_t = sm.tile([P, 1], f32)
        nc.vector.memset(eps_t, LN_EPS)
        # warm the sigmoid activation table early (overlaps with DMA)
        dmy = sm.tile([1, 1], f32)
        nc.vector.memset(dmy, 0.0)
        dmy2 = sm.tile([1, 1], f32)
        nc.scalar.activation(dmy2, dmy, AF.Sigmoid)

        # ---- input DMAs: x first on SP FIFO, then weights in use order ----
        x_sb = []
        for t in range(NT):
            xt = xp.tile([P, C], f32, tag="x_sb", name=f"x_sb{t}")
            nc.sync.dma_start(out=xt, in_=x[t * P : (t + 1) * P, :])
            x_sb.append(xt)
        w1_sb = wp.tile([P, KC, CR], wdt)
        nc.sync.dma_start(out=w1_sb, in_=w1.rearrange("(a p) c -> p a c", p=P))
        w2_sb = wp.tile([P, 2, C], wdt)
        nc.sync.dma_start(out=w2_sb[:, 0, :], in_=w2[0:P, :])
        nc.sync.dma_start(out=w2_sb[0:64, 1, :], in_=w2[P:CR, :])
        wv_sb = wp.tile([P, KC, C], wdt)
        nc.sync.dma_start(out=wv_sb, in_=wv.rearrange("(a p) c -> p a c", p=P))
        wo_sb = wp.tile([P, KC, C], wdt)
        nc.sync.dma_start(out=wo_sb, in_=wo.rearrange("(a p) c -> p a c", p=P))
        if not fast:
            vec_sb = sm.tile([1, 6, C], f32)
            nc.sync.dma_start(out=vec_sb, in_=vecs[:, :, :])

        # ---- per-tile: bf16 cast (ACT) for column sums; row sums (DVE) ----
        xb_sb = []
        xsum_all = sm.tile([P, NT], f32, name="xsum_all") if fast else None
        for t in range(NT):
            xb = xp.tile([P, C], bf16, tag="xb_sb", name=f"xb_sb{t}")
            if fast:
                if t == NT - 1:
                    # last tile gates gc: cast on DVE (2x mode, faster)
                    nc.vector.tensor_scalar(
                        xb, x_sb[t], 1.0, 0.0, op0=OP.mult, op1=OP.add,
                        accum_out=xsum_all[:, t : t + 1],
                    )
                else:
                    nc.scalar.activation(
                        xb, x_sb[t], AF.Copy, accum_out=xsum_all[:, t : t + 1]
                    )
            else:
                nc.scalar.copy(xb, x_sb[t])
            xb_sb.append(xb)

        # ---- gc = mean_n x  (column sums via PE; ones stationary) ----
        cs_ps = [pp.tile([1, H], f32, tag="rowps", name=f"cs_ps{h}") for h in range(NH)]
        for t in range(NT):
            for h in range(NH):
                nc.tensor.matmul(
                    cs_ps[h],
                    ones_col,
                    xb_sb[t][:, h * H : (h + 1) * H],
                    start=(t == 0),
                    stop=(t == NT - 1),
                )
        gc_sb = sm.tile([1, C], wdt)
        nc.scalar.activation(gc_sb[0:1, 0:H], cs_ps[0], AF.Copy, bias=0.0, scale=s_gc)
        nc.vector.tensor_scalar_mul(gc_sb[0:1, H : 2 * H], cs_ps[1], s_gc)

        # ---- transpose gc to partitions: gcT[:, j] = gc[128j:128j+128] ----
        gcT_ps = pc.tile([P, 8], f32, tag="colps")
        for j in range(KC):
            nc.tensor.matmul(
                gcT_ps[:, j : j + 1], gc_sb[0:1, j * P : (j + 1) * P], one11,
                start=True, stop=True,
            )
        gcT_sb = sm.tile([P, KC], wdt)
        nc.vector.tensor_copy(gcT_sb, gcT_ps[:, 0:KC])

        # ---- h1 = relu(gc @ W1 (+ b1)) ----
        h1_ps = pp.tile([1, CR], f32, tag="rowps")
        for j in range(KC):
            nc.tensor.matmul(
                h1_ps, gcT_sb[:, j : j + 1], w1_sb[:, j, :],
                start=(j == 0), stop=(j == KC - 1),
            )
        vb_ps = [pp.tile([1, H], f32, tag="rowps", name=f"vb_ps{h}") for h in range(NH)]
        for j in range(KC):
            nc.tensor.matmul(
                vb_ps[0],
                gcT_sb[:, j : j + 1],
                wv_sb[:, j, 0:H],
                start=(j == 0),
                stop=(j == KC - 1),
            )
        h1r = sm.tile([1, CR], wdt)
        if fast:
            nc.scalar.activation(h1r, h1_ps, AF.Relu, scale=s_h1)
        else:
            h1f = sm.tile([1, CR], f32)
            nc.vector.tensor_add(h1f, h1_ps, vec_sb[0:1, 1, 0:CR])
            nc.vector.tensor_scalar_max(h1r, h1f, 0.0)

        # ---- transpose h1 (192 = 128 + 64) ----
        h1T_ps = pc.tile([P, 8], f32, tag="colps")
        nc.tensor.matmul(h1T_ps[:, 0:1], h1r[0:1, 0:P], one11, start=True, stop=True)
        nc.tensor.matmul(
            h1T_ps[0:64, 1:2], h1r[0:1, P:CR], one11, start=True, stop=True
        )
        h1T_sb = sm.tile([P, 2], wdt)
        nc.vector.tensor_copy(h1T_sb[:, 0:1], h1T_ps[:, 0:1])
        nc.vector.tensor_copy(h1T_sb[0:64, 1:2], h1T_ps[0:64, 1:2])

        # ---- ca = sigmoid(h1 @ W2 (+ b2)) ----
        ca_ps = [pp.tile([1, H], f32, tag="rowps", name=f"ca_ps{h}") for h in range(NH)]
        for h in range(NH):
            sl = slice(h * H, (h + 1) * H)
            nc.tensor.matmul(
                ca_ps[h], h1T_sb[:, 0:1], w2_sb[:, 0, sl], start=True, stop=False
            )
            nc.tensor.matmul(
                ca_ps[h], h1T_sb[0:64, 1:2], w2_sb[0:64, 1, sl],
                start=False, stop=True,
            )
        ca_sb = sm.tile([1, C], f32)
        if fast:
            for h in range(NH):
                sl = slice(h * H, (h + 1) * H)
                nc.scalar.activation(ca_sb[0:1, sl], ca_ps[h], AF.Sigmoid, scale=s_ca)
        else:
            cap_f = sm.tile([1, C], f32)
            for h in range(NH):
                sl = slice(h * H, (h + 1) * H)
                nc.vector.tensor_add(cap_f[0:1, sl], ca_ps[h], vec_sb[0:1, 2, sl])
            nc.scalar.activation(ca_sb, cap_f, AF.Sigmoid)

        # ---- vbar = gc @ Wv (+ bv) ----
        vbar_sb = sm.tile([1, C], f32)
        if fast:
            nc.scalar.activation(
                vbar_sb[0:1, 0:H], vb_ps[0], AF.Copy, bias=0.0, scale=s_vb
            )
        else:
            nc.vector.tensor_add(vbar_sb[0:1, 0:H], vb_ps[0], vec_sb[0:1, 0, 0:H])

        for j in range(KC):
            nc.tensor.matmul(
                vb_ps[1],
                gcT_sb[:, j : j + 1],
                wv_sb[:, j, H : 2 * H],
                start=(j == 0),
                stop=(j == KC - 1),
            )
        if fast:
            nc.vector.tensor_scalar_mul(vbar_sb[0:1, H : 2 * H], vb_ps[1], s_vb)
        else:
            nc.vector.tensor_add(
                vbar_sb[0:1, H : 2 * H], vb_ps[1], vec_sb[0:1, 0, H : 2 * H]
            )

        # ---- g = vbar * ca ; transpose ; o = g @ Wo — in halves ----
        g_sb = sm.tile([1, C], wdt)
        gT_ps = pc.tile([P, 8], f32, tag="colps")
        gT_sb = sm.tile([P, KC], wdt)
        o_ps = [pp.tile([1, H], f32, tag="rowps", name=f"o_ps{h}") for h in range(NH)]
        for half in range(2):
            hs = slice(half * H, (half + 1) * H)
            if use_fp8:
                nc.vector.scalar_tensor_tensor(
                    out=g_sb[0:1, hs], in0=vbar_sb[0:1, hs], scalar=s_g,
                    in1=ca_sb[0:1, hs], op0=OP.mult, op1=OP.mult,
                )
            else:
                nc.vector.tensor_mul(
                    g_sb[0:1, hs], vbar_sb[0:1, hs], ca_sb[0:1, hs]
                )
            for j in range(half * 3, half * 3 + 3):
                nc.tensor.matmul(
                    gT_ps[:, j : j + 1], g_sb[0:1, j * P : (j + 1) * P], one11,
                    start=True, stop=True,
                )
            nc.vector.tensor_copy(
                gT_sb[:, half * 3 : half * 3 + 3],
                gT_ps[:, half * 3 : half * 3 + 3],
            )
            for j in range(half * 3, half * 3 + 3):
                for h in range(NH):
                    nc.tensor.matmul(
                        o_ps[h],
                        gT_sb[:, j : j + 1],
                        wo_sb[:, j, h * H : (h + 1) * H],
                        start=(j == 0),
                        stop=(j == KC - 1),
                    )

        o_sb = sm.tile([1, C], bf16 if fast else f32)
        if fast:
            nc.scalar.activation(o_sb[0:1, 0:H], o_ps[0], AF.Copy, bias=0.0, scale=s_o)
            nc.vector.tensor_scalar_mul(o_sb[0:1, H : 2 * H], o_ps[1], s_o)
        else:
            for h in range(NH):
                sl = slice(h * H, (h + 1) * H)
                nc.vector.tensor_add(o_sb[0:1, sl], o_ps[h], vec_sb[0:1, 3, sl])

        # ---- broadcast o across partitions via K=1 matmul ----
        obc_sb = sm.tile([P, C], f32)
        for h in range(NH):
            sl = slice(h * H, (h + 1) * H)
            obp = po.tile([P, H], f32, tag="obc", name=f"obp{h}")
            nc.tensor.matmul(obp, ones_row, o_sb[0:1, sl], start=True, stop=True)
            nc.scalar.copy(obc_sb[:, sl], obp)

        if fast:
            # sum(o) on one partition, then broadcast to (128,1) via K=1 matmul
            osum_row = sm.tile([1, 1], f32)
            nc.vector.tensor_reduce(osum_row, o_sb, AX.X, OP.add)
            osum_row_b = sm.tile([1, 1], bf16)
            nc.vector.tensor_copy(osum_row_b, osum_row)
            osb_ps = po.tile([P, 1], f32, tag="obc")
            nc.tensor.matmul(osb_ps, ones_row, osum_row_b, start=True, stop=True)
            osum = sm.tile([P, 1], f32)
            nc.vector.tensor_copy(osum, osb_ps)

            # batched per-tile stats: mu, mu^2, (eps - mu^2)
            mu_all = sm.tile([P, NT], f32)
            nc.vector.tensor_scalar(
                mu_all, xsum_all, osum, 1.0 / C, op0=OP.add, op1=OP.mult
            )
            musq_all = sm.tile([P, NT], f32)
            nc.vector.tensor_mul(musq_all, mu_all, mu_all)
            em_all = sm.tile([P, NT], f32)
            nc.vector.tensor_scalar(
                em_all, musq_all, -1.0, LN_EPS, op0=OP.mult, op1=OP.add
            )

            for t in range(NT):
                on_dve = t >= NT - N_DVE_TILES
                u = up.tile([P, C], f32, tag="u", name=f"u{t}")
                # u = x + o on DVE or Pool; mu folded into the final scale op
                if on_dve:
                    nc.vector.scalar_tensor_tensor(
                        out=u, in0=x_sb[t], scalar=0.0, in1=obc_sb,
                        op0=OP.bypass, op1=OP.add,
                    )
                else:
                    nc.gpsimd.tensor_add(u, x_sb[t], obc_sb)
                usq = sq.tile([P, C], f32, tag="usq")
                uss = s8.tile([P, 1], f32, tag="uss", name=f"uss{t}")
                nc.scalar.activation(usq, u, AF.Square, accum_out=uss)
                # std = sqrt(uss/C + eps - mu^2)
                std = st.tile([P, 1], f32, tag="std")
                nc.scalar.activation(
                    std, uss, AF.Sqrt, bias=em_all[:, t : t + 1], scale=1.0 / C
                )
                rstd = st.tile([P, 1], f32, tag="rstd")
                nc.vector.reciprocal(rstd, std)
                nc.vector.tensor_scalar(
                    u, u, mu_all[:, t : t + 1], rstd, op0=OP.subtract, op1=OP.mult
                )
                nc.scalar.dma_start(out=y[t * P : (t + 1) * P, :], in_=u)
        else:
            gamma_bc = sm.tile([P, C], f32)
            beta_bc = sm.tile([P, C], f32)
            for h in range(NH):
                sl = slice(h * H, (h + 1) * H)
                gbp = po.tile([P, H], f32, tag="obc", name=f"gbp{h}")
                nc.tensor.matmul(
                    gbp, ones_row, vec_sb[0:1, 4, sl], start=True, stop=True
                )
                nc.vector.tensor_copy(gamma_bc[:, sl], gbp)
            for h in range(NH):
                sl = slice(h * H, (h + 1) * H)
                bbp = po.tile([P, H], f32, tag="obc", name=f"bbp{h}")
                nc.tensor.matmul(
                    bbp, ones_row, vec_sb[0:1, 5, sl], start=True, stop=True
                )
                nc.vector.tensor_copy(beta_bc[:, sl], bbp)

            for t in range(NT):
                z = x_sb[t]
                nc.gpsimd.tensor_add(z, z, obc_sb)
                stats = sq.tile([P, 3, 6], f32, tag="stats", name=f"stats{t}")
                zg = z.rearrange("p (s d) -> p s d", s=3)
                for s in range(3):
                    nc.vector.bn_stats(stats[:, s, :], zg[:, s, :])
                mv = st.tile([P, 2], f32, tag="mv")
                nc.vector.bn_aggr(mv, stats)
                std = st.tile([P, 1], f32, tag="std")
                nc.scalar.activation(std, mv[:, 1:2], AF.Sqrt, bias=eps_t)
                rstd = st.tile([P, 1], f32, tag="rstd")
                nc.vector.reciprocal(rstd, std)
                zq = up.tile([P, C], f32, tag="u")
                nc.vector.scalar_tensor_tensor(
                    out=zq, in0=z, scalar=mv[:, 0:1], in1=gamma_bc,
                    op0=OP.subtract, op1=OP.mult,
                )
                nc.vector.tensor_scalar_mul(zq, zq, rstd)
                nc.vector.tensor_add(zq, zq, beta_bc)
                nc.scalar.dma_start(out=y[t * P : (t + 1) * P, :], in_=zq)

    nc.compile()
    return nc




def _build_general():
    return _build(False)


def _get_nc(fast: bool):
    key = ("nc", fast)
    if key not in _CACHE:
        _CACHE[key] = _build_fast() if fast else _build_general()
    return _CACHE[key]


def _pack(w, rows, wdt, scale):
    """Pack a (rows, cols) weight as [128, ceil(rows/128), cols] fp8/bf16."""
    a = -(-rows // P)
    out = np.zeros((P, a, w.shape[1]), np.float32)
    wf = np.asarray(w, np.float32) * scale
    for j in range(a):
        r = wf[j * P : (j + 1) * P]
        out[: r.shape[0], j] = r
    return np.ascontiguousarray(out.astype(wdt))


def make_in_maps(x, Wv, bv, W1, b1, W2, b2, Wo, bo, gamma, beta, fast=True):
    if FP8 and fast:
        import concourse.mybir as mybir

        wdt = mybir.dt.np(mybir.dt.float8e4)
        s = SW
    else:
        wdt = ml_dtypes.bfloat16
        s = 1.0
    if fast:
        ident = np.eye(P, dtype=ml_dtypes.bfloat16)
        w1p = _pack(W1, C, wdt, s).reshape(P, KC * CR)
        w2p = _pack(W2, CR, wdt, s).reshape(P, 2 * C)
        idp = np.ascontiguousarray(ident).view(np.uint8).view(wdt)
        wop = _pack(Wo, C, wdt, s)
        shared = {
            "blob": np.ascontiguousarray(np.concatenate([w1p, w2p, idp], axis=1)),
            "wv": _pack(Wv, C, wdt, s),
            "wo_a": np.ascontiguousarray(wop[:, :, 0:H]),
            "wo_b": np.ascontiguousarray(wop[:, :, H:C]),
        }
    else:
        shared = {
            "wv": np.ascontiguousarray(np.asarray(Wv, np.float32).astype(wdt)),
            "w1": np.ascontiguousarray(np.asarray(W1, np.float32).astype(wdt)),
            "w2": np.ascontiguousarray(np.asarray(W2, np.float32).astype(wdt)),
            "wo": np.ascontiguousarray(np.asarray(Wo, np.float32).astype(wdt)),
        }
        b1p = np.zeros(C, np.float32)
        b1p[:CR] = np.asarray(b1, np.float32)
        vecs = np.stack(
            [
                np.asarray(bv, np.float32),
                b1p,
                np.asarray(b2, np.float32),
                np.asarray(bo, np.float32),
                np.asarray(gamma, np.float32),
                np.asarray(beta, np.float32),
            ]
        )
        shared["vecs"] = np.ascontiguousarray(vecs.reshape(1, 6, C))
    return [
        {"x": np.ascontiguousarray(np.asarray(x[i], np.float32)), **shared}
        for i in range(NCORES)
    ]


def _is_fast(inputs):
    def z(a):
        return bool(np.all(np.asarray(a) == 0.0))

    return (
        bool(np.all(np.asarray(inputs["gamma"]) == 1.0))
        and z(inputs["beta"]) and z(inputs["bv"]) and z(inputs["b1"])
        and z(inputs["b2"]) and z(inputs["bo"])
    )


def run(inputs, trace=False, **kw):
    from concourse.bass_utils import run_bass_kernel_spmd

    fast = _is_fast(inputs)
    nc = _get_nc(fast)
    in_maps = make_in_maps(
        inputs["x"], inputs["Wv"], inputs["bv"], inputs["W1"], inputs["b1"],
        inputs["W2"], inputs["b2"], inputs["Wo"], inputs["bo"],
        inputs["gamma"], inputs["beta"], fast=fast,
    )
    res = run_bass_kernel_spmd(nc, in_maps, list(range(NCORES)), trace=trace, **kw)
    out = np.stack([r["y"] for r in res.results]).astype(np.float32)
    return out, res


def kernel(
    x, Wq, bq, Wk, bk, Wv, bv, W1, b1, W2, b2, Wo, bo, gamma, beta
) -> np.ndarray:
    # Wq/bq/Wk/bk provably do not affect the output (uniform softmax).
    out, _ = run(
        dict(
            x=x, Wv=Wv, bv=bv, W1=W1, b1=b1, W2=W2, b2=b2, Wo=Wo, bo=bo,
            gamma=gamma, beta=beta,
        )
    )
    return out
